# revision 41
# baseline (speedup 1.0000x reference)
"""LongAxisSelfAttention Trainium2 kernel (8-core SPMD, Bass/Tile).

Problem: B=2, S=4096, H=768, 12 heads x 64: heads 0-5 full attention,
heads 6-11 4-way strided ("axis") attention.

Sharding (uniform SPMD program, data-parameterized per core):
  core c: batch b=c//4, ci=c%4.
    full heads  F = [0,1,2] if ci<2 else [3,4,5], q-half qh=ci%2
    axis heads  A = [6,7,8] if ci<2 else [9,10,11], groups (0,1) or (2,3)

v3 design (cost-model driven; ScalarE exp is the ~250us bottleneck):
  - Projections: bf16 matmuls (full PE rate; fp8 quantization of the
    768-deep contraction was measured too coarse for the 2e-2 gate).
    Outputs packed in 128-row pairs of 64-row head blocks. Scale/bias
    epilogues on DVE keep ScalarE free for exp.
  - Attention: scoresT[k,q] = K.T @ Q (bf16), exp on ScalarE
    (scale=0.125 folded in, bf16 out), ctx accumulated Q-MAJOR:
    stationary = exp tile [128k,128q], moving = [V|1] [128k,65] bf16 ->
    ctx[q, d|sum] at full PE rate with PSUM k-accumulation. No output
    transpose; DVE does reciprocal + normalize straight from PSUM.
  - Emission is software-pipelined: unit-0 q0/q1 attention rides inside
    the projection chunk loop (k-availability synced, chunk tails
    deferred into the next chunk), and axis projections are sliced into
    thunks spread across full-attention q-tiles.
"""

import numpy as np

B, S, H = 2, 4096, 768
NH, D, SEG = 12, 64, 6
P = 128
KT_H = H // P            # 6 hidden k-tiles
QHALF = S // 2           # 2048
AXLEN = S // 2           # per-core axis length (2 groups x 1024)
GLEN = S // 4            # 1024
CH = 1024                # full-proj chunk cols
CH2 = 512                # axis-proj chunk cols

_CACHE = {}

# static per-unit slice tables: (base_partition, m_tile)
K_SLOT = [(0, 0), (64, 0), (0, 1)]          # K in kvT_sb / Q in qT_sb
Q_SLOT = [(0, 0), (64, 0), (0, 1)]
VT_SLOT = [(64, 1), (0, 2), (64, 2)]        # V transpose source in kvT_sb
AXK_SLOT = [(0, 0), (64, 0), (0, 2)]        # in axT_sb
AXQ_SLOT = [(0, 1), (64, 1), (0, 3)]
AXVT_SLOT = [(64, 2), (64, 3), (0, 4)]


def _build_nc():
    import concourse.bass as bass
    import concourse.mybir as mybir
    import concourse.tile as tile
    from concourse import bacc
    from contextlib import ExitStack

    F32 = mybir.dt.float32
    BF16 = mybir.dt.bfloat16
    AF = mybir.ActivationFunctionType
    MUL = mybir.AluOpType.mult
    ADD = mybir.AluOpType.add

    nc = bacc.Bacc(None, target_bir_lowering=False)

    # ---- DRAM I/O ----
    # h16[p,t,s] = hidden[s, 128t+p]  (bf16, q-half-permuted cols)
    h16 = nc.dram_tensor("h16", [P, KT_H, S], BF16, kind="ExternalInput")
    hax16 = nc.dram_tensor("hax16", [P, KT_H, AXLEN], BF16, kind="ExternalInput")
    # w16[p,t,64*b+j] = Wblk[b][j, 128t+p]; 18 out-64-blocks in order:
    # [kf0,kf1,kf2,vf0,vf1,vf2, qf0,qf1,qf2, ka0,ka1,qa0,qa1,ka2,va0,qa2,va1,va2]
    w16 = nc.dram_tensor("w16", [P, KT_H, 18 * 64], BF16, kind="ExternalInput")
    # bias pairs (128 rows): (0,1)(2,3)(4,5)(6,7)(8,z)(9,10)(11,12)(13,14)(15,16)(17,z)
    b_all = nc.dram_tensor("b_all", [1280], F32, kind="ExternalInput")
    ident_b = nc.dram_tensor("ident_b", [P, P], BF16, kind="ExternalInput")
    out_full = nc.dram_tensor("out_full", [QHALF, 192], F32, kind="ExternalOutput")
    out_ax = nc.dram_tensor("out_ax", [AXLEN, 192], F32, kind="ExternalOutput")

    with tile.TileContext(nc) as tc, ExitStack() as top:
        constp = top.enter_context(tc.tile_pool(name="constp", bufs=1))
        persist = top.enter_context(tc.tile_pool(name="persist", bufs=1))
        hpool = top.enter_context(tc.tile_pool(name="hpool", bufs=2))
        exps = top.enter_context(tc.tile_pool(name="exps", bufs=42))
        epi = top.enter_context(tc.tile_pool(name="epi", bufs=3))
        # PSUM (bank-padded slots): pps 2x1 = 2 banks, scp 2x2 = 4,
        # ctxp 2x1 = 2 -> exactly 8 banks. V-transposes borrow scp slots
        # (same tag).
        pps = top.enter_context(tc.tile_pool(name="pps", bufs=2, space="PSUM"))
        scp = top.enter_context(tc.tile_pool(name="scp", bufs=2, space="PSUM"))
        ctxp = top.enter_context(tc.tile_pool(name="ctxp", bufs=2, space="PSUM"))

        # DMA order tuned for lead-in: bias + the first projections' weight
        # slices (Q pairs, K0 pair) go first; the rest interleaves with the
        # first h chunk below.
        bias_sb = constp.tile([P, 10], F32)
        nc.sync.dma_start(bias_sb[:], b_all.rearrange("(m p) -> p m", p=P))
        w_sb = constp.tile([P, KT_H, 18 * 64], BF16)
        nc.sync.dma_start(w_sb[:, :, 384:576], w16[:, :, 384:576])  # Q blocks
        nc.sync.dma_start(w_sb[:, :, 0:128], w16[:, :, 0:128])      # K0 pair
        id_b = constp.tile([P, P], BF16)

        # persistent projection outputs (bf16)
        kvT_sb = persist.tile([P, 3, S], BF16)        # 24KB/part
        qT_sb = persist.tile([P, 2, QHALF], BF16)     # 8KB
        axT_sb = persist.tile([P, 5, AXLEN], BF16)    # 20KB
        v_sb = persist.tile([P, S // P, 3, 65], BF16)      # 12.2KB
        vax_sb = persist.tile([P, AXLEN // P, 3, 65], BF16)  # 6.1KB

        # ---------- helpers ----------
        def proj_pair(c0, ncol2, rhs, g2, dst, bias_m):
            """One out-block pair (w16 cols c0..c0+ncol2) x 512 cols: 6 bf16
            matmuls + DVE scale/bias epilogue (GpSimd cannot access PSUM)."""
            ps = pps.tile([P, 512], F32, name="projps", tag="projps")
            for t in range(KT_H):
                nc.tensor.matmul(
                    ps[0:ncol2, :],
                    w_sb[:, t, c0:c0 + ncol2],
                    rhs[:, t, g2 * 512:(g2 + 1) * 512],
                    start=(t == 0), stop=(t == KT_H - 1),
                )
            nc.vector.tensor_scalar(
                dst, ps[0:ncol2, :], 1.0,
                bias_sb[0:ncol2, bias_m:bias_m + 1], op0=MUL, op1=ADD)

        def v_transposes(src_sb, slot_tab, dst_v, cc0, nblk, units=range(3)):
            """Transpose nblk P-blocks of the units' V rows into dst_v."""
            for u in units:
                base, mt = slot_tab[u]
                tp = scp.tile([P, 16, 64], BF16, name="vtr", tag="scps")
                for j in range(nblk):
                    cc = cc0 + j
                    nc.tensor.transpose(
                        tp[:, j, :],
                        src_sb[base:base + 64, mt, cc * P:(cc + 1) * P],
                        id_b[base:base + 64, base:base + 64],
                    )
                nc.vector.tensor_copy(
                    dst_v[:, cc0:cc0 + nblk, u, 0:64], tp[:, 0:nblk, :])

        def score_exp(kT, qT, q_sl, k0, kp):
            """Scores + exp for one (q-512, k-256) piece; returns exp tile."""
            sc = scp.tile([P, 2, 512], F32, name="scps", tag="scps")
            ex = exps.tile([P, 2, 512], BF16, name="exsb", tag="exsb")
            for h in range(2):
                ksl = slice(k0 + kp * 256 + h * P, k0 + kp * 256 + (h + 1) * P)
                nc.tensor.matmul(sc[:, h, :], kT[:, ksl], qT[:, q_sl],
                                 start=True, stop=True)
            nc.scalar.activation(ex[:], sc[:], AF.Exp, scale=0.125)
            return ex

        def ctx_accum(ex, vsb, u, kc0, kp, ctx_ps, npair):
            for h in range(2):
                kc = kc0 + kp * 2 + h
                for j in range(4):
                    nc.tensor.matmul(
                        ctx_ps[:, j, :],
                        ex[:, h, j * P:(j + 1) * P],
                        vsb[:, kc, u, :],
                        start=(kp == 0 and h == 0),
                        stop=(kp == npair - 1 and h == 1),
                    )

        def attn_qtile(kT, qT, vsb, u, k0, q_sl, nk, ctx_ps):
            npair = nk // 256
            exs = [score_exp(kT, qT, q_sl, k0, kp) for kp in range(npair)]
            for j in range(4):
                for kp in range(npair):
                    for h in range(2):
                        kc = k0 // P + kp * 2 + h
                        nc.tensor.matmul(
                            ctx_ps[:, j, :],
                            exs[kp][:, h, j * P:(j + 1) * P],
                            vsb[:, kc, u, :],
                            start=(kp == 0 and h == 0),
                            stop=(kp == npair - 1 and h == 1),
                        )

        def attn_epilogue(ctx_ps, outv, ocol, c0):
            recip = epi.tile([P, 4], F32, name="recip", tag="recip")
            nc.vector.reciprocal(recip[:], ctx_ps[:, :, 64])
            outsb = epi.tile([P, 4, 64], F32, name="outsb", tag="outsb")
            nc.vector.tensor_tensor(
                outsb[:], ctx_ps[:, :, 0:64],
                recip[:, :, None].to_broadcast([P, 4, 64]), op=MUL)
            nc.sync.dma_start(outv[:, c0:c0 + 4, ocol:ocol + 64], outsb[:])

        outv_full = out_full.rearrange("(c p) o -> p c o", p=P)
        outv_ax = out_ax.rearrange("(c p) o -> p c o", p=P)

        # ---------- phase A: full projections + pipelined u0 q0/q1 ----------
        kb0, km0 = K_SLOT[0]
        kT_u0 = kvT_sb[kb0:kb0 + 64, km0, :]
        qb0, qm0 = Q_SLOT[0]
        qT_u0 = qT_sb[qb0:qb0 + 64, qm0, :]
        # exp tiles for u0 q0/q1, collected during the chunk loop; their
        # (j-serial) ctx runs at the phase-A flush
        ex_u0 = {qi: [None] * (S // 256) for qi in range(3)}

        # Chunk-tail work (V transposes, ones) is deferred into the NEXT
        # chunk's projection stream so the next chunk's scores aren't
        # serialized behind it on PE.
        tailq = []

        def pop_tail(n):
            for _ in range(n):
                if tailq:
                    tailq.pop(0)()

        def mk_tail(ch):
            thunks = [lambda u=u: v_transposes(kvT_sb, VT_SLOT, v_sb,
                                               ch * 8, 8, units=[u])
                      for u in range(3)]
            thunks.append(lambda: nc.gpsimd.memset(
                v_sb[:, ch * 8:(ch + 1) * 8, :, 64:65], 1.0))
            return thunks

        def u0_piece(kp, qi):
            ex_u0[qi][kp] = score_exp(
                kT_u0, qT_u0, slice(qi * 512, (qi + 1) * 512), 0, kp)

        hch_tiles = {}

        def h_dma(ch):
            hch = hpool.tile([P, KT_H, CH], BF16, name="hch", tag="hch")
            # split halves so the first projections wait on less data
            for hf in range(2):
                sl = slice(ch * CH + hf * 512, ch * CH + (hf + 1) * 512)
                nc.sync.dma_start(hch[:, :, hf * 512:(hf + 1) * 512],
                                  h16[:, :, sl])
            hch_tiles[ch] = hch

        # first chunk: interleave the remaining kv weights between the two
        # h halves so neither blocks the other's consumers
        hch0 = hpool.tile([P, KT_H, CH], BF16, name="hch", tag="hch")
        nc.sync.dma_start(hch0[:, :, 0:512], h16[:, :, 0:512])
        nc.sync.dma_start(w_sb[:, :, 128:384], w16[:, :, 128:384])
        nc.sync.dma_start(hch0[:, :, 512:1024], h16[:, :, 512:1024])
        nc.sync.dma_start(id_b[:], ident_b[:])
        hch_tiles[0] = hch0
        for ch in range(S // CH):
            if ch + 1 < S // CH:
                h_dma(ch + 1)
            hch = hch_tiles.pop(ch)
            for g2 in range(CH // 512):
                kpa = ch * 4 + g2 * 2
                kpb = kpa + 1
                col = slice(ch * CH + g2 * 512, ch * CH + (g2 + 1) * 512)
                # q first (both pipelined q-tiles' Q cols land in ch0;
                # qi1 pieces for ch0/g2=0 are deferred until g2=1)
                if ch < QHALF // CH:
                    proj_pair(384, P, hch, g2, qT_sb[:, 0, col], 3)
                proj_pair(0, P, hch, g2, kvT_sb[:, 0, col], 0)
                pop_tail(2)
                u0_piece(kpa, 0)
                if ch > 0 or g2 > 0:
                    u0_piece(kpa, 1)
                proj_pair(128, P, hch, g2, kvT_sb[:, 1, col], 1)
                pop_tail(1)
                u0_piece(kpb, 0)
                proj_pair(256, P, hch, g2, kvT_sb[:, 2, col], 2)
                pop_tail(1)
                if ch < QHALF // CH:
                    proj_pair(512, 64, hch, g2, qT_sb[0:64, 1, col], 4)
                if ch > 0 or g2 > 0:
                    u0_piece(kpb, 1)
                if ch == 0 and g2 == 1:
                    # qi1 pieces for g2=0's kps, now that Q0[512:1024] exists
                    u0_piece(0, 1)
                    u0_piece(1, 1)
                if ch >= 1 and g2 == 1:
                    # partial u0 q2 pre-pieces (SBUF-limited to 8 total)
                    kps = (ch * 4, ch * 4 + 1) + (
                        (ch * 4 + 2,) if ch < 3 else ())
                    for kp in kps:
                        u0_piece(kp, 2)
            tailq.extend(mk_tail(ch))

        pop_tail(len(tailq))
        nc.sync.dma_start(w_sb[:, :, 576:1152], w16[:, :, 576:1152])

        # ---------- attention: piece-level lag pipeline over all q-tiles ----
        # axis projections, sliced into thunks spread across the
        # full-attention q-tiles (fills PE slack without starving ACT)
        hax_tiles = {}

        def ax_dma(axch):
            hax = hpool.tile([P, KT_H, CH2], BF16, name="haxch", tag="haxch")
            nc.sync.dma_start(hax[:], hax16[:, :, axch * CH2:(axch + 1) * CH2])
            hax_tiles[axch] = hax

        def ax_proj(axch, m):
            hax = hax_tiles[axch]
            col = slice(axch * CH2, (axch + 1) * CH2)
            # ax pairs: [Ka0|Ka1] [Qa0|Qa1] [Ka2|Va0] [Qa2|Va1] [Va2|-]
            if m < 4:
                proj_pair(576 + 128 * m, P, hax, 0, axT_sb[:, m, col], 5 + m)
            else:
                proj_pair(1088, 64, hax, 0, axT_sb[0:64, 4, col], 9)

        def ax_tail(axch, u):
            v_transposes(axT_sb, AXVT_SLOT, vax_sb, axch * 4, 4, units=[u])
            if u == 2:
                nc.gpsimd.memset(
                    vax_sb[:, axch * 4:(axch + 1) * 4, :, 64:65], 1.0)

        ax_work = []
        for axch in range(4):
            ax_work.append(lambda axch=axch: ax_dma(axch))
            for m in range(5):
                ax_work.append(lambda axch=axch, m=m: ax_proj(axch, m))
            for u in range(3):
                ax_work.append(lambda axch=axch, u=u: ax_tail(axch, u))

        def pop_ax(n):
            for _ in range(n):
                if ax_work:
                    ax_work.pop(0)()

        def ctx_flush(exs, vsb, u, k0, outv, ocol, c0):
            """j-serial ctx accumulation (one open PSUM group at a time:
            HW start/stop cannot interleave groups in a bank) + epilogue."""
            npair = len(exs)
            ctx_ps = ctxp.tile([P, 4, 65], F32, name="ctxps", tag="ctxps",
                               padded_shape=[P, 4, 128])
            for j in range(4):
                for kp in range(npair):
                    for h in range(2):
                        kc = k0 // P + kp * 2 + h
                        nc.tensor.matmul(
                            ctx_ps[:, j, :],
                            exs[kp][:, h, j * P:(j + 1) * P],
                            vsb[:, kc, u, :],
                            start=(kp == 0 and h == 0),
                            stop=(kp == npair - 1 and h == 1),
                        )
            attn_epilogue(ctx_ps, outv, ocol, c0)

        # jobs: (kT, qT, vsb, u, k0, q0_abs, outv, ocol, c0, nk, n_ax_pop)
        jobs = []
        for u in range(3):
            kb, km = K_SLOT[u]
            qb, qm = Q_SLOT[u]
            for qs in range(2 if u == 0 else 0, QHALF // 512):
                jobs.append((kvT_sb[kb:kb + 64, km, :],
                             qT_sb[qb:qb + 64, qm, :], v_sb, u, 0,
                             qs * 512, outv_full, u * 64, qs * 4, S, 4))
        for u in range(3):
            kb, km = AXK_SLOT[u]
            qb, qm = AXQ_SLOT[u]
            for g in range(2):
                for qs in range(GLEN // 512):
                    jobs.append((axT_sb[kb:kb + 64, km, :],
                                 axT_sb[qb:qb + 64, qm, :], vax_sb, u,
                                 g * GLEN, g * GLEN + qs * 512, outv_ax,
                                 u * 64, (g * GLEN) // P + qs * 4, GLEN, 0))

        # prime ACT with the first main-loop scores, then flush the
        # pipelined u0 q0/q1 ctx, then run the lag-1 pipeline
        pend = None  # (exs, vsb, u, k0, outv, ocol, c0)

        def emit_job(job, phaseA_flush=False, pre=None):
            nonlocal pend
            kT, qT, vsb, u, k0, q0a, outv, ocol, c0, nk, nax = job
            q_sl = slice(q0a, q0a + 512)
            npair = nk // 256

            def piece(kp):
                if pre is not None and pre[kp] is not None:
                    return pre[kp]
                return score_exp(kT, qT, q_sl, k0, kp)

            if phaseA_flush:
                # flush first: releases the 32 pipelined exp tiles so the
                # pre-scores below can allocate (ACT still has phase-A
                # backlog, so no priming is needed here)
                ctx_flush(ex_u0[0], v_sb, 0, 0, outv_full, 0, 0)
                ctx_flush(ex_u0[1], v_sb, 0, 0, outv_full, 0, 4)
            npre = min(6, npair)
            exs = [piece(kp) for kp in range(npre)]
            if pend is not None:
                ctx_flush(*pend)
            exs += [piece(kp) for kp in range(npre, npair)]
            pend = (exs, vsb, u, k0, outv, ocol, c0)
            pop_ax(nax)

        for i, job in enumerate(jobs):
            emit_job(job, phaseA_flush=(i == 0),
                     pre=(ex_u0[2] if i == 0 else None))
        pop_ax(len(ax_work))
        ctx_flush(*pend)

    nc.finalize()
    return nc


def _get_nc():
    if "nc" not in _CACHE:
        _CACHE["nc"] = _build_nc()
    return _CACHE["nc"]


def _pack_bf16(rows_t, bf):
    """[768, n] f32 -> [128, 6, n] bf16 with dim0 = 128t + p."""
    r = rows_t.reshape(KT_H, P, -1).transpose(1, 0, 2)
    return np.ascontiguousarray(r.astype(bf))


def _prep_inputs(hidden_states, Wq, bq, Wk, bk, Wv, bv):
    """Build the 8 per-core input maps (host-side marshalling)."""
    import ml_dtypes
    bf = ml_dtypes.bfloat16

    hs = np.ascontiguousarray(hidden_states, dtype=np.float32)
    eye = np.eye(P, dtype=np.float32).astype(bf)
    in_maps = []
    for c in range(8):
        b, ci = divmod(c, 4)
        F0 = 0 if ci < 2 else 3          # first full head
        A0 = 6 if ci < 2 else 9          # first axis head
        qh = ci % 2
        ga, gb = (0, 1) if ci % 2 == 0 else (2, 3)

        hb = hs[b]                        # [S, H]
        hperm = np.concatenate([hb[qh * QHALF:(qh + 1) * QHALF],
                                hb[(1 - qh) * QHALF:(2 - qh) * QHALF]], axis=0)
        h16 = _pack_bf16(hperm.T, bf)
        hax = np.concatenate([hb[ga::4], hb[gb::4]], axis=0)
        hax16 = _pack_bf16(hax.T, bf)

        def rows(W, h0, i):
            return W[64 * (h0 + i):64 * (h0 + i) + 64]

        blocks = [rows(Wk, F0, 0), rows(Wk, F0, 1), rows(Wk, F0, 2),
                  rows(Wv, F0, 0), rows(Wv, F0, 1), rows(Wv, F0, 2),
                  rows(Wq, F0, 0), rows(Wq, F0, 1), rows(Wq, F0, 2),
                  rows(Wk, A0, 0), rows(Wk, A0, 1),
                  rows(Wq, A0, 0), rows(Wq, A0, 1),
                  rows(Wk, A0, 2), rows(Wv, A0, 0),
                  rows(Wq, A0, 2), rows(Wv, A0, 1), rows(Wv, A0, 2)]
        # w16[p, t, 64b+j] = blocks[b][j, 128t+p]
        wcat = np.concatenate(blocks, axis=0)            # [1152, 768]
        w16 = np.ascontiguousarray(
            wcat.T.reshape(KT_H, P, 1152).transpose(1, 0, 2).astype(bf))

        def brows(bvec, h0, i):
            return bvec[64 * (h0 + i):64 * (h0 + i) + 64]

        z64b = np.zeros(64, np.float32)
        b_kv = np.concatenate([brows(bk, F0, 0), brows(bk, F0, 1),
                               brows(bk, F0, 2), brows(bv, F0, 0),
                               brows(bv, F0, 1), brows(bv, F0, 2)])
        b_q = np.concatenate([brows(bq, F0, 0), brows(bq, F0, 1),
                              brows(bq, F0, 2), z64b])
        b_ax = np.concatenate([brows(bk, A0, 0), brows(bk, A0, 1),
                               brows(bq, A0, 0), brows(bq, A0, 1),
                               brows(bk, A0, 2), brows(bv, A0, 0),
                               brows(bq, A0, 2), brows(bv, A0, 1),
                               brows(bv, A0, 2), z64b])
        b_all = np.concatenate([b_kv, b_q, b_ax]).astype(np.float32)

        in_maps.append({
            "h16": h16, "hax16": hax16, "w16": w16,
            "b_all": b_all, "ident_b": eye,
        })
    return in_maps


def _assemble(results):
    out = np.empty((B, S, H), np.float32)
    for c in range(8):
        b, ci = divmod(c, 4)
        F0 = 0 if ci < 2 else 3
        A0 = 6 if ci < 2 else 9
        qh = ci % 2
        ga, gb = (0, 1) if ci % 2 == 0 else (2, 3)
        r = results[c]
        out[b, qh * QHALF:(qh + 1) * QHALF, 64 * F0:64 * F0 + 192] = r["out_full"]
        out[b, ga::4, 64 * A0:64 * A0 + 192] = r["out_ax"][:GLEN]
        out[b, gb::4, 64 * A0:64 * A0 + 192] = r["out_ax"][GLEN:]
    return out


def run(inputs, trace=False):
    from concourse.bass_utils import run_bass_kernel_spmd
    nc = _get_nc()
    in_maps = _prep_inputs(**inputs)
    res = run_bass_kernel_spmd(nc, in_maps, core_ids=list(range(8)), trace=trace)
    return _assemble(res.results), res


def kernel(**inputs):
    out, _ = run(inputs, trace=False)
    return out


# revision 46
# speedup vs baseline: 1.0070x; 1.0070x over previous
"""LongAxisSelfAttention Trainium2 kernel (8-core SPMD, Bass/Tile).

Problem: B=2, S=4096, H=768, 12 heads x 64: heads 0-5 full attention,
heads 6-11 4-way strided ("axis") attention.

Sharding (uniform SPMD program, data-parameterized per core):
  core c: batch b=c//4, ci=c%4.
    full heads  F = [0,1,2] if ci<2 else [3,4,5], q-half qh=ci%2
    axis heads  A = [6,7,8] if ci<2 else [9,10,11], groups (0,1) or (2,3)

v3 design (cost-model driven; ScalarE exp is the ~250us bottleneck):
  - Projections: bf16 matmuls (full PE rate; fp8 quantization of the
    768-deep contraction was measured too coarse for the 2e-2 gate).
    Outputs packed in 128-row pairs of 64-row head blocks. Scale/bias
    epilogues on DVE keep ScalarE free for exp.
  - Attention: scoresT[k,q] = K.T @ Q (bf16), exp on ScalarE
    (scale=0.125 folded in, bf16 out), ctx accumulated Q-MAJOR:
    stationary = exp tile [128k,128q], moving = [V|1] [128k,65] bf16 ->
    ctx[q, d|sum] at full PE rate with PSUM k-accumulation. No output
    transpose; DVE does reciprocal + normalize straight from PSUM.
  - Emission is software-pipelined: unit-0 q0/q1 attention rides inside
    the projection chunk loop (k-availability synced, chunk tails
    deferred into the next chunk), and axis projections are sliced into
    thunks spread across full-attention q-tiles.
"""

import numpy as np

B, S, H = 2, 4096, 768
NH, D, SEG = 12, 64, 6
P = 128
KT_H = H // P            # 6 hidden k-tiles
QHALF = S // 2           # 2048
AXLEN = S // 2           # per-core axis length (2 groups x 1024)
GLEN = S // 4            # 1024
CH = 1024                # full-proj chunk cols
CH2 = 512                # axis-proj chunk cols

_CACHE = {}

# static per-unit slice tables: (base_partition, m_tile)
K_SLOT = [(0, 0), (64, 0), (0, 1)]          # K in kvT_sb / Q in qT_sb
Q_SLOT = [(0, 0), (64, 0), (0, 1)]
VT_SLOT = [(64, 1), (0, 2), (64, 2)]        # V transpose source in kvT_sb
AXK_SLOT = [(0, 0), (64, 0), (0, 2)]        # in axT_sb
AXQ_SLOT = [(0, 1), (64, 1), (0, 3)]
AXVT_SLOT = [(64, 2), (64, 3), (0, 4)]


def _build_nc():
    import concourse.bass as bass
    import concourse.mybir as mybir
    import concourse.tile as tile
    from concourse import bacc
    from contextlib import ExitStack

    F32 = mybir.dt.float32
    BF16 = mybir.dt.bfloat16
    AF = mybir.ActivationFunctionType
    MUL = mybir.AluOpType.mult
    ADD = mybir.AluOpType.add

    nc = bacc.Bacc(None, target_bir_lowering=False)

    # ---- DRAM I/O ----
    # h16[p,t,s] = hidden[s, 128t+p]  (bf16, q-half-permuted cols)
    h16 = nc.dram_tensor("h16", [P, KT_H, S], BF16, kind="ExternalInput")
    hax16 = nc.dram_tensor("hax16", [P, KT_H, AXLEN], BF16, kind="ExternalInput")
    # w16[p,t,64*b+j] = Wblk[b][j, 128t+p]; 18 out-64-blocks in order:
    # [kf0,kf1,kf2,vf0,vf1,vf2, qf0,qf1,qf2, ka0,ka1,qa0,qa1,ka2,va0,qa2,va1,va2]
    w16 = nc.dram_tensor("w16", [P, KT_H, 18 * 64], BF16, kind="ExternalInput")
    # bias pairs (128 rows): (0,1)(2,3)(4,5)(6,7)(8,z)(9,10)(11,12)(13,14)(15,16)(17,z)
    b_all = nc.dram_tensor("b_all", [1280], F32, kind="ExternalInput")
    ident_b = nc.dram_tensor("ident_b", [P, P], BF16, kind="ExternalInput")
    out_full = nc.dram_tensor("out_full", [QHALF, 192], F32, kind="ExternalOutput")
    out_ax = nc.dram_tensor("out_ax", [AXLEN, 192], F32, kind="ExternalOutput")

    with tile.TileContext(nc) as tc, ExitStack() as top:
        constp = top.enter_context(tc.tile_pool(name="constp", bufs=1))
        persist = top.enter_context(tc.tile_pool(name="persist", bufs=1))
        hpool = top.enter_context(tc.tile_pool(name="hpool", bufs=2))
        exps = top.enter_context(tc.tile_pool(name="exps", bufs=42))
        epi = top.enter_context(tc.tile_pool(name="epi", bufs=3))
        # PSUM (bank-padded slots): pps 2x1 = 2 banks, scp 2x2 = 4,
        # ctxp 2x1 = 2 -> exactly 8 banks. V-transposes borrow scp slots
        # (same tag).
        pps = top.enter_context(tc.tile_pool(name="pps", bufs=2, space="PSUM"))
        scp = top.enter_context(tc.tile_pool(name="scp", bufs=2, space="PSUM"))
        ctxp = top.enter_context(tc.tile_pool(name="ctxp", bufs=2, space="PSUM"))

        # DMA order tuned for lead-in: bias + the first projections' weight
        # slices (Q pairs, K0 pair) go first; the rest interleaves with the
        # first h chunk below.
        bias_sb = constp.tile([P, 10], F32)
        nc.sync.dma_start(bias_sb[:], b_all.rearrange("(m p) -> p m", p=P))
        w_sb = constp.tile([P, KT_H, 18 * 64], BF16)
        nc.sync.dma_start(w_sb[:, :, 384:576], w16[:, :, 384:576])  # Q blocks
        nc.sync.dma_start(w_sb[:, :, 0:128], w16[:, :, 0:128])      # K0 pair
        id_b = constp.tile([P, P], BF16)

        # persistent projection outputs (bf16)
        kvT_sb = persist.tile([P, 3, S], BF16)        # 24KB/part
        qT_sb = persist.tile([P, 2, QHALF], BF16)     # 8KB
        axT_sb = persist.tile([P, 5, AXLEN], BF16)    # 20KB
        v_sb = persist.tile([P, S // P, 3, 65], BF16)      # 12.2KB
        vax_sb = persist.tile([P, AXLEN // P, 3, 65], BF16)  # 6.1KB

        # ---------- helpers ----------
        def proj_pair(c0, ncol2, rhs, g2, dst, bias_m):
            """One out-block pair (w16 cols c0..c0+ncol2) x 512 cols: 6 bf16
            matmuls + DVE scale/bias epilogue (GpSimd cannot access PSUM)."""
            ps = pps.tile([P, 512], F32, name="projps", tag="projps")
            for t in range(KT_H):
                nc.tensor.matmul(
                    ps[0:ncol2, :],
                    w_sb[:, t, c0:c0 + ncol2],
                    rhs[:, t, g2 * 512:(g2 + 1) * 512],
                    start=(t == 0), stop=(t == KT_H - 1),
                )
            nc.vector.tensor_scalar(
                dst, ps[0:ncol2, :], 1.0,
                bias_sb[0:ncol2, bias_m:bias_m + 1], op0=MUL, op1=ADD)

        def v_transposes(src_sb, slot_tab, dst_v, cc0, nblk, units=range(3)):
            """Transpose nblk P-blocks of the units' V rows into dst_v."""
            for u in units:
                base, mt = slot_tab[u]
                tp = scp.tile([P, 16, 64], BF16, name="vtr", tag="scps")
                for j in range(nblk):
                    cc = cc0 + j
                    nc.tensor.transpose(
                        tp[:, j, :],
                        src_sb[base:base + 64, mt, cc * P:(cc + 1) * P],
                        id_b[base:base + 64, base:base + 64],
                    )
                nc.vector.tensor_copy(
                    dst_v[:, cc0:cc0 + nblk, u, 0:64], tp[:, 0:nblk, :])

        def score_exp(kT, qT, q_sl, k0, kp):
            """Scores + exp for one (q-512, k-256) piece; returns exp tile."""
            sc = scp.tile([P, 2, 512], F32, name="scps", tag="scps")
            ex = exps.tile([P, 2, 512], BF16, name="exsb", tag="exsb")
            for h in range(2):
                ksl = slice(k0 + kp * 256 + h * P, k0 + kp * 256 + (h + 1) * P)
                nc.tensor.matmul(sc[:, h, :], kT[:, ksl], qT[:, q_sl],
                                 start=True, stop=True)
            nc.scalar.activation(ex[:], sc[:], AF.Exp, scale=0.125)
            return ex

        def ctx_accum(ex, vsb, u, kc0, kp, ctx_ps, npair):
            for h in range(2):
                kc = kc0 + kp * 2 + h
                for j in range(4):
                    nc.tensor.matmul(
                        ctx_ps[:, j, :],
                        ex[:, h, j * P:(j + 1) * P],
                        vsb[:, kc, u, :],
                        start=(kp == 0 and h == 0),
                        stop=(kp == npair - 1 and h == 1),
                    )

        def attn_qtile(kT, qT, vsb, u, k0, q_sl, nk, ctx_ps):
            npair = nk // 256
            exs = [score_exp(kT, qT, q_sl, k0, kp) for kp in range(npair)]
            for j in range(4):
                for kp in range(npair):
                    for h in range(2):
                        kc = k0 // P + kp * 2 + h
                        nc.tensor.matmul(
                            ctx_ps[:, j, :],
                            exs[kp][:, h, j * P:(j + 1) * P],
                            vsb[:, kc, u, :],
                            start=(kp == 0 and h == 0),
                            stop=(kp == npair - 1 and h == 1),
                        )

        def attn_epilogue(ctx_ps, outv, ocol, c0):
            recip = epi.tile([P, 4], F32, name="recip", tag="recip")
            nc.vector.reciprocal(recip[:], ctx_ps[:, :, 64])
            outsb = epi.tile([P, 4, 64], F32, name="outsb", tag="outsb")
            nc.vector.tensor_tensor(
                outsb[:], ctx_ps[:, :, 0:64],
                recip[:, :, None].to_broadcast([P, 4, 64]), op=MUL)
            nc.sync.dma_start(outv[:, c0:c0 + 4, ocol:ocol + 64], outsb[:])

        outv_full = out_full.rearrange("(c p) o -> p c o", p=P)
        outv_ax = out_ax.rearrange("(c p) o -> p c o", p=P)

        # ---------- phase A: full projections + pipelined u0 q0/q1 ----------
        kb0, km0 = K_SLOT[0]
        kT_u0 = kvT_sb[kb0:kb0 + 64, km0, :]
        qb0, qm0 = Q_SLOT[0]
        qT_u0 = qT_sb[qb0:qb0 + 64, qm0, :]
        # exp tiles for u0 q0/q1, collected during the chunk loop; their
        # (j-serial) ctx runs at the phase-A flush
        ex_u0 = {qi: [None] * (S // 256) for qi in range(3)}

        # Chunk-tail work (V transposes, ones) is deferred into the NEXT
        # chunk's projection stream so the next chunk's scores aren't
        # serialized behind it on PE.
        tailq = []

        def pop_tail(n):
            for _ in range(n):
                if tailq:
                    tailq.pop(0)()

        def mk_tail(ch):
            thunks = [lambda u=u: v_transposes(kvT_sb, VT_SLOT, v_sb,
                                               ch * 8, 8, units=[u])
                      for u in range(3)]
            thunks.append(lambda: nc.gpsimd.memset(
                v_sb[:, ch * 8:(ch + 1) * 8, :, 64:65], 1.0))
            return thunks

        def u0_piece(kp, qi):
            ex_u0[qi][kp] = score_exp(
                kT_u0, qT_u0, slice(qi * 512, (qi + 1) * 512), 0, kp)

        hch_tiles = {}

        def h_dma(ch):
            hch = hpool.tile([P, KT_H, CH], BF16, name="hch", tag="hch")
            # split halves so the first projections wait on less data
            for hf in range(2):
                sl = slice(ch * CH + hf * 512, ch * CH + (hf + 1) * 512)
                nc.sync.dma_start(hch[:, :, hf * 512:(hf + 1) * 512],
                                  h16[:, :, sl])
            hch_tiles[ch] = hch

        # first chunk: interleave the remaining kv weights between the two
        # h halves so neither blocks the other's consumers
        hch0 = hpool.tile([P, KT_H, CH], BF16, name="hch", tag="hch")
        nc.sync.dma_start(hch0[:, :, 0:512], h16[:, :, 0:512])
        nc.sync.dma_start(w_sb[:, :, 128:384], w16[:, :, 128:384])
        nc.sync.dma_start(hch0[:, :, 512:1024], h16[:, :, 512:1024])
        nc.sync.dma_start(id_b[:], ident_b[:])
        hch_tiles[0] = hch0
        for ch in range(S // CH):
            if ch + 1 < S // CH:
                h_dma(ch + 1)
            hch = hch_tiles.pop(ch)
            for g2 in range(CH // 512):
                kpa = ch * 4 + g2 * 2
                kpb = kpa + 1
                col = slice(ch * CH + g2 * 512, ch * CH + (g2 + 1) * 512)
                # q first (both pipelined q-tiles' Q cols land in ch0;
                # qi1 pieces for ch0/g2=0 are deferred until g2=1)
                if ch < QHALF // CH:
                    proj_pair(384, P, hch, g2, qT_sb[:, 0, col], 3)
                proj_pair(0, P, hch, g2, kvT_sb[:, 0, col], 0)
                pop_tail(2)
                u0_piece(kpa, 0)
                if ch > 0 or g2 > 0:
                    u0_piece(kpa, 1)
                proj_pair(128, P, hch, g2, kvT_sb[:, 1, col], 1)
                pop_tail(1)
                u0_piece(kpb, 0)
                proj_pair(256, P, hch, g2, kvT_sb[:, 2, col], 2)
                pop_tail(1)
                if ch < QHALF // CH:
                    proj_pair(512, 64, hch, g2, qT_sb[0:64, 1, col], 4)
                if ch > 0 or g2 > 0:
                    u0_piece(kpb, 1)
                if ch == 0 and g2 == 1:
                    # qi1 pieces for g2=0's kps, now that Q0[512:1024] exists
                    u0_piece(0, 1)
                    u0_piece(1, 1)
                if ch >= 1 and g2 == 1:
                    # partial u0 q2 pre-pieces (SBUF-limited to 8 total)
                    kps = (ch * 4, ch * 4 + 1) + (
                        (ch * 4 + 2,) if ch < 3 else ())
                    for kp in kps:
                        u0_piece(kp, 2)
            tailq.extend(mk_tail(ch))

        pop_tail(len(tailq))
        nc.sync.dma_start(w_sb[:, :, 576:1152], w16[:, :, 576:1152])

        # ---------- attention: piece-level lag pipeline over all q-tiles ----
        # axis projections, sliced into thunks spread across the
        # full-attention q-tiles (fills PE slack without starving ACT)
        hax_tiles = {}

        def ax_dma(axch):
            hax = hpool.tile([P, KT_H, CH2], BF16, name="haxch", tag="haxch")
            nc.sync.dma_start(hax[:], hax16[:, :, axch * CH2:(axch + 1) * CH2])
            hax_tiles[axch] = hax

        def ax_proj(axch, m):
            hax = hax_tiles[axch]
            col = slice(axch * CH2, (axch + 1) * CH2)
            # ax pairs: [Ka0|Ka1] [Qa0|Qa1] [Ka2|Va0] [Qa2|Va1] [Va2|-]
            if m < 4:
                proj_pair(576 + 128 * m, P, hax, 0, axT_sb[:, m, col], 5 + m)
            else:
                proj_pair(1088, 64, hax, 0, axT_sb[0:64, 4, col], 9)

        def ax_tail(axch, u):
            v_transposes(axT_sb, AXVT_SLOT, vax_sb, axch * 4, 4, units=[u])
            if u == 2:
                nc.gpsimd.memset(
                    vax_sb[:, axch * 4:(axch + 1) * 4, :, 64:65], 1.0)

        ax_work = []
        for axch in range(4):
            ax_work.append(lambda axch=axch: ax_dma(axch))
            for m in range(5):
                ax_work.append(lambda axch=axch, m=m: ax_proj(axch, m))
            for u in range(3):
                ax_work.append(lambda axch=axch, u=u: ax_tail(axch, u))

        def pop_ax(n):
            for _ in range(n):
                if ax_work:
                    ax_work.pop(0)()

        def ctx_flush(exs, vsb, u, k0, outv, ocol, c0):
            """j-serial ctx accumulation (one open PSUM group at a time:
            HW start/stop cannot interleave groups in a bank) + epilogue."""
            npair = len(exs)
            ctx_ps = ctxp.tile([P, 4, 65], F32, name="ctxps", tag="ctxps",
                               padded_shape=[P, 4, 128])
            for j in range(4):
                for kp in range(npair):
                    for h in range(2):
                        kc = k0 // P + kp * 2 + h
                        nc.tensor.matmul(
                            ctx_ps[:, j, :],
                            exs[kp][:, h, j * P:(j + 1) * P],
                            vsb[:, kc, u, :],
                            start=(kp == 0 and h == 0),
                            stop=(kp == npair - 1 and h == 1),
                        )
            attn_epilogue(ctx_ps, outv, ocol, c0)

        # jobs: (kT, qT, vsb, u, k0, q0_abs, outv, ocol, c0, nk, n_ax_pop)
        jobs = []
        for u in range(3):
            kb, km = K_SLOT[u]
            qb, qm = Q_SLOT[u]
            for qs in range(2 if u == 0 else 0, QHALF // 512):
                jobs.append((kvT_sb[kb:kb + 64, km, :],
                             qT_sb[qb:qb + 64, qm, :], v_sb, u, 0,
                             qs * 512, outv_full, u * 64, qs * 4, S, 4))
        for u in range(3):
            kb, km = AXK_SLOT[u]
            qb, qm = AXQ_SLOT[u]
            for g in range(2):
                for qs in range(GLEN // 512):
                    jobs.append((axT_sb[kb:kb + 64, km, :],
                                 axT_sb[qb:qb + 64, qm, :], vax_sb, u,
                                 g * GLEN, g * GLEN + qs * 512, outv_ax,
                                 u * 64, (g * GLEN) // P + qs * 4, GLEN, 0))

        # prime ACT with the first main-loop scores, then flush the
        # pipelined u0 q0/q1 ctx, then run the lag-1 pipeline
        pend = None  # (exs, vsb, u, k0, outv, ocol, c0)

        def emit_job(job, phaseA_flush=False, pre=None):
            nonlocal pend
            kT, qT, vsb, u, k0, q0a, outv, ocol, c0, nk, nax = job
            q_sl = slice(q0a, q0a + 512)
            npair = nk // 256

            def piece(kp):
                if pre is not None and pre[kp] is not None:
                    return pre[kp]
                return score_exp(kT, qT, q_sl, k0, kp)

            if phaseA_flush:
                # q0's flush releases 16 exp slots (40 of 42 live here), so
                # the pre-scores below can allocate; q1's flush then runs
                # with those scores already feeding ScalarE
                ctx_flush(ex_u0[0], v_sb, 0, 0, outv_full, 0, 0)
            npre = min(6, npair)
            exs = [piece(kp) for kp in range(npre)]
            if phaseA_flush:
                ctx_flush(ex_u0[1], v_sb, 0, 0, outv_full, 0, 4)
            if pend is not None:
                ctx_flush(*pend)
            exs += [piece(kp) for kp in range(npre, npair)]
            pend = (exs, vsb, u, k0, outv, ocol, c0)
            pop_ax(nax)

        for i, job in enumerate(jobs):
            emit_job(job, phaseA_flush=(i == 0),
                     pre=(ex_u0[2] if i == 0 else None))
        pop_ax(len(ax_work))
        ctx_flush(*pend)

    nc.finalize()
    return nc


def _get_nc():
    if "nc" not in _CACHE:
        _CACHE["nc"] = _build_nc()
    return _CACHE["nc"]


def _pack_bf16(rows_t, bf):
    """[768, n] f32 -> [128, 6, n] bf16 with dim0 = 128t + p."""
    r = rows_t.reshape(KT_H, P, -1).transpose(1, 0, 2)
    return np.ascontiguousarray(r.astype(bf))


def _prep_inputs(hidden_states, Wq, bq, Wk, bk, Wv, bv):
    """Build the 8 per-core input maps (host-side marshalling)."""
    import ml_dtypes
    bf = ml_dtypes.bfloat16

    hs = np.ascontiguousarray(hidden_states, dtype=np.float32)
    eye = np.eye(P, dtype=np.float32).astype(bf)
    in_maps = []
    for c in range(8):
        b, ci = divmod(c, 4)
        F0 = 0 if ci < 2 else 3          # first full head
        A0 = 6 if ci < 2 else 9          # first axis head
        qh = ci % 2
        ga, gb = (0, 1) if ci % 2 == 0 else (2, 3)

        hb = hs[b]                        # [S, H]
        hperm = np.concatenate([hb[qh * QHALF:(qh + 1) * QHALF],
                                hb[(1 - qh) * QHALF:(2 - qh) * QHALF]], axis=0)
        h16 = _pack_bf16(hperm.T, bf)
        hax = np.concatenate([hb[ga::4], hb[gb::4]], axis=0)
        hax16 = _pack_bf16(hax.T, bf)

        def rows(W, h0, i):
            return W[64 * (h0 + i):64 * (h0 + i) + 64]

        blocks = [rows(Wk, F0, 0), rows(Wk, F0, 1), rows(Wk, F0, 2),
                  rows(Wv, F0, 0), rows(Wv, F0, 1), rows(Wv, F0, 2),
                  rows(Wq, F0, 0), rows(Wq, F0, 1), rows(Wq, F0, 2),
                  rows(Wk, A0, 0), rows(Wk, A0, 1),
                  rows(Wq, A0, 0), rows(Wq, A0, 1),
                  rows(Wk, A0, 2), rows(Wv, A0, 0),
                  rows(Wq, A0, 2), rows(Wv, A0, 1), rows(Wv, A0, 2)]
        # w16[p, t, 64b+j] = blocks[b][j, 128t+p]
        wcat = np.concatenate(blocks, axis=0)            # [1152, 768]
        w16 = np.ascontiguousarray(
            wcat.T.reshape(KT_H, P, 1152).transpose(1, 0, 2).astype(bf))

        def brows(bvec, h0, i):
            return bvec[64 * (h0 + i):64 * (h0 + i) + 64]

        z64b = np.zeros(64, np.float32)
        b_kv = np.concatenate([brows(bk, F0, 0), brows(bk, F0, 1),
                               brows(bk, F0, 2), brows(bv, F0, 0),
                               brows(bv, F0, 1), brows(bv, F0, 2)])
        b_q = np.concatenate([brows(bq, F0, 0), brows(bq, F0, 1),
                              brows(bq, F0, 2), z64b])
        b_ax = np.concatenate([brows(bk, A0, 0), brows(bk, A0, 1),
                               brows(bq, A0, 0), brows(bq, A0, 1),
                               brows(bk, A0, 2), brows(bv, A0, 0),
                               brows(bq, A0, 2), brows(bv, A0, 1),
                               brows(bv, A0, 2), z64b])
        b_all = np.concatenate([b_kv, b_q, b_ax]).astype(np.float32)

        in_maps.append({
            "h16": h16, "hax16": hax16, "w16": w16,
            "b_all": b_all, "ident_b": eye,
        })
    return in_maps


def _assemble(results):
    out = np.empty((B, S, H), np.float32)
    for c in range(8):
        b, ci = divmod(c, 4)
        F0 = 0 if ci < 2 else 3
        A0 = 6 if ci < 2 else 9
        qh = ci % 2
        ga, gb = (0, 1) if ci % 2 == 0 else (2, 3)
        r = results[c]
        out[b, qh * QHALF:(qh + 1) * QHALF, 64 * F0:64 * F0 + 192] = r["out_full"]
        out[b, ga::4, 64 * A0:64 * A0 + 192] = r["out_ax"][:GLEN]
        out[b, gb::4, 64 * A0:64 * A0 + 192] = r["out_ax"][GLEN:]
    return out


def run(inputs, trace=False):
    from concourse.bass_utils import run_bass_kernel_spmd
    nc = _get_nc()
    in_maps = _prep_inputs(**inputs)
    res = run_bass_kernel_spmd(nc, in_maps, core_ids=list(range(8)), trace=trace)
    return _assemble(res.results), res


def kernel(**inputs):
    out, _ = run(inputs, trace=False)
    return out
